# revision 4
# baseline (speedup 1.0000x reference)
"""Trainium2 Bass kernel: AdapterController hard-routing MoE.

Per (router m, batch b): e = expert_index[m, b], then
  u[m, b] = swish(x[b] @ Wd[m, e] + bd[m, e]) @ Wu[m, e].

Strategy (8 NeuronCores): data-parallel over batch (B == 8, one batch row
per core). The expert gather is done host-side -- each core only needs its
4 selected (Wd, bd, Wu) triples, not the full expert tables. Routers are
packed in pairs so the down-projection runs as full 128-wide matmuls:

  down: lhsT = packed Wd chunk [c128, 128]   (two routers' [c,64] concat)
        rhs  = xT chunk        [c128, s512]
        -> psum zT [128, s512] accumulated over 8 c-chunks
  bias + swish fused on ScalarE (Silu activation, per-partition bias)
  up:   row-packed K=64 matmuls (router A on partitions 0-63, B on 64-127)
        lhsT = zT[64r:64r+64, s128], rhs = Wu[64r:64r+64, c512]
        -> psum u [s128, c512]
  psum -> SBUF f32 staging (DVE/ACT alternating), 1 MiB DMAs out.
"""

import numpy as np
import ml_dtypes
from contextlib import ExitStack

import concourse.bass as bass
import concourse.bacc as bacc
import concourse.tile as tile
from concourse import mybir
from concourse.bass_utils import run_bass_kernel_spmd

M_ROUTERS = 4
C = 1024
D = 64
B = 8
S = 2048
NCORES = 8
NPAIR = M_ROUTERS // 2   # routers packed two per 128-wide matmul
KCH = C // 128           # 8 contraction chunks for the down matmul
ST_DOWN = 512            # moving free dim for down matmuls (1 PSUM bank)
N_ST = S // ST_DOWN      # 4
TB = 128                 # token block (up matmul stationary M)
N_TB = S // TB           # 16
CC = 512                 # c chunk for up matmuls (1 PSUM bank)
N_CC = C // CC           # 2
BLK = 2                  # token blocks per staging tile -> 1 MiB output DMAs

BF16 = mybir.dt.bfloat16
F32 = mybir.dt.float32

_GRAPH = None


def _build(reps=1):
    nc = bacc.Bacc(None, target_bir_lowering=False, debug=False)
    xt = nc.declare_dram_parameter("xt", [KCH, 128, S], BF16, isOutput=False)
    wd = nc.declare_dram_parameter("wd", [NPAIR, 128, KCH, 128], BF16, isOutput=False)
    wu = nc.declare_dram_parameter("wu", [NPAIR, 128, C], BF16, isOutput=False)
    bi = nc.declare_dram_parameter("bias", [NPAIR, 128, 1], F32, isOutput=False)
    out = nc.declare_dram_parameter("out", [M_ROUTERS, S, C], F32, isOutput=True)

    with ExitStack() as ctx:
        tc = ctx.enter_context(tile.TileContext(nc))
        const = ctx.enter_context(tc.tile_pool(name="const", bufs=1))
        psum_d = ctx.enter_context(tc.tile_pool(name="psum_d", bufs=2, space="PSUM"))
        psum_u = ctx.enter_context(tc.tile_pool(name="psum_u", bufs=3, space="PSUM"))
        stage = ctx.enter_context(tc.tile_pool(name="stage", bufs=4))

        if reps > 1:
            # benchmark variant: serialize `reps` full executions; the Tile
            # For_i back-edge is an all-engine barrier, so each iteration
            # times like an independent single-shot run.
            loop = ctx.enter_context(tc.For_i(0, reps, 1))

        x_sb = []
        for k in range(KCH):
            t = const.tile([128, S], BF16, tag=f"x{k}")
            nc.gpsimd.dma_start(t[:], xt[k])
            x_sb.append(t)
        wd_sb, wu_sb, bi_sb, z_sb = [], [], [], []
        for p in range(NPAIR):
            t = const.tile([128, KCH, 128], BF16, tag=f"wd{p}")
            nc.gpsimd.dma_start(t[:, :, :], wd[p])
            wd_sb.append(t)
            t = const.tile([128, C], BF16, tag=f"wu{p}")
            nc.gpsimd.dma_start(t[:], wu[p])
            wu_sb.append(t)
            t = const.tile([128, 1], F32, tag=f"bi{p}")
            nc.gpsimd.dma_start(t[:], bi[p])
            bi_sb.append(t)
            t = const.tile([128, S], BF16, tag=f"z{p}")
            z_sb.append(t)

        ncopy = 0
        ndma = 0
        for p in range(NPAIR):
            # down projection + bias + swish for router pair p
            for st in range(N_ST):
                zp = psum_d.tile([128, ST_DOWN], F32, tag="zp")
                for k in range(KCH):
                    nc.tensor.matmul(
                        zp[:],
                        wd_sb[p][:, k, :],
                        x_sb[k][:, st * ST_DOWN:(st + 1) * ST_DOWN],
                        start=(k == 0),
                        stop=(k == KCH - 1),
                    )
                nc.scalar.activation(
                    z_sb[p][:, st * ST_DOWN:(st + 1) * ST_DOWN],
                    zp[:],
                    mybir.ActivationFunctionType.Silu,
                    bias=bi_sb[p][:],
                )
            # up projection, both routers of the pair interleaved so their
            # K=64 matmuls land on disjoint PE row groups and run packed
            for t0 in range(N_TB // BLK):
                stgs = [stage.tile([128, BLK, C], F32, tag=f"stg{r}", name=f"stg{r}") for r in range(2)]
                for a in range(BLK):
                    j = t0 * BLK + a
                    for r in range(2):
                        lo, hi = 64 * r, 64 * (r + 1)
                        ups = psum_u.tile([128, C], F32, tag="ups")
                        for cc in range(N_CC):
                            nc.tensor.matmul(
                                ups[:, cc * CC:(cc + 1) * CC],
                                z_sb[p][lo:hi, j * TB:(j + 1) * TB],
                                wu_sb[p][lo:hi, cc * CC:(cc + 1) * CC],
                                start=True,
                                stop=True,
                            )
                        if ncopy % 2 == 0:
                            nc.vector.tensor_copy(stgs[r][:, a, :], ups[:])
                        else:
                            nc.scalar.copy(stgs[r][:, a, :], ups[:])
                        ncopy += 1
                for r in range(2):
                    m = 2 * p + r
                    dma_eng = nc.sync if ndma % 2 == 0 else nc.scalar
                    dma_eng.dma_start(
                        out[m, t0 * BLK * TB:(t0 + 1) * BLK * TB, :]
                        .rearrange("(a q) c -> q a c", q=128),
                        stgs[r][:, :, :],
                    )
                    ndma += 1

    nc.finalize()
    return nc


def _get_graph(reps=1):
    global _GRAPH
    if reps != 1:
        return _build(reps)
    if _GRAPH is None:
        _GRAPH = _build()
    return _GRAPH


def _pack_core_inputs(b, x, dw, db, uw, ei):
    """Host-side shard + expert-gather + layout packing for core b."""
    sel = ei[:, b]                                   # [M]
    ar = np.arange(M_ROUTERS)
    wd_sel = dw[ar, sel]                             # [M, C, D]
    bi_sel = db[ar, sel]                             # [M, D]
    wu_sel = uw[ar, sel]                             # [M, D, C]

    # down weights: concat router pair along D -> [C, 128], then chunk C
    # and make the within-chunk c index the leading (partition) dim.
    wd_packed = np.stack(
        [np.concatenate([wd_sel[2 * p], wd_sel[2 * p + 1]], axis=1) for p in range(NPAIR)]
    )                                                # [NPAIR, C, 128]
    wd_packed = (
        wd_packed.reshape(NPAIR, KCH, 128, 128)
        .transpose(0, 2, 1, 3)                       # [NPAIR, 128(c), KCH, 128(d2)]
        .astype(ml_dtypes.bfloat16)
    )
    bi_packed = np.stack(
        [np.concatenate([bi_sel[2 * p], bi_sel[2 * p + 1]]) for p in range(NPAIR)]
    ).reshape(NPAIR, 128, 1).astype(np.float32)
    wu_packed = np.stack(
        [np.concatenate([wu_sel[2 * p], wu_sel[2 * p + 1]], axis=0) for p in range(NPAIR)]
    ).astype(ml_dtypes.bfloat16)                     # [NPAIR, 128(d2), C]

    xtb = np.ascontiguousarray(x[b].T).astype(ml_dtypes.bfloat16)  # [C, S]
    xtb = xtb.reshape(KCH, 128, S)

    return {
        "xt": xtb,
        "wd": np.ascontiguousarray(wd_packed),
        "wu": np.ascontiguousarray(wu_packed),
        "bias": bi_packed,
    }


def _run(inputs, trace=False):
    x = np.asarray(inputs["x"], dtype=np.float32)
    dw = np.asarray(inputs["down_samplers_weights"], dtype=np.float32)
    db = np.asarray(inputs["down_samplers_bias"], dtype=np.float32)
    uw = np.asarray(inputs["up_samplers_weights"], dtype=np.float32)
    ei = np.asarray(inputs["expert_index"]).astype(np.int64)

    nc = _get_graph()
    in_maps = [_pack_core_inputs(b, x, dw, db, uw, ei) for b in range(NCORES)]
    res = run_bass_kernel_spmd(nc, in_maps, core_ids=list(range(NCORES)), trace=trace)
    out = np.stack([res.results[i]["out"] for i in range(NCORES)], axis=1)
    return out, res


def kernel(x, down_samplers_weights, down_samplers_bias, up_samplers_weights,
           expert_index):
    out, _ = _run(
        {
            "x": x,
            "down_samplers_weights": down_samplers_weights,
            "down_samplers_bias": down_samplers_bias,
            "up_samplers_weights": up_samplers_weights,
            "expert_index": expert_index,
        },
        trace=False,
    )
    return out


# revision 20
# speedup vs baseline: 6.1083x; 6.1083x over previous
"""Trainium2 Bass kernel: AdapterController hard-routing MoE.

Per (router m, batch b): e = expert_index[m, b], then
  u[m, b] = swish(x[b] @ Wd[m, e] + bd[m, e]) @ Wu[m, e].

Strategy (8 NeuronCores): data-parallel over batch (B == 8, one batch row
per core). The expert gather is done host-side -- each core only needs its
4 selected (Wd, bd, Wu) triples, not the full expert tables. Routers are
packed in pairs so the down-projection runs as full 128-wide matmuls:

  down: lhsT = packed Wd chunk [c128, 128]   (two routers' [c,64] concat)
        rhs  = xT chunk        [c128, s512]
        -> psum zT [128, s512] accumulated over 8 c-chunks
  bias + swish fused on ScalarE (Silu activation, per-partition bias)
  up:   row-packed K=64 matmuls (router A on partitions 0-63, B on 64-127)
        lhsT = zT[64r:64r+64, s128], rhs = Wu[64r:64r+64, c512]
        -> psum u [s128, c512]
  psum -> SBUF f32 staging (DVE/ACT alternating), 1 MiB DMAs out.
"""

import numpy as np
import ml_dtypes
from contextlib import ExitStack

import concourse.bass as bass
import concourse.bacc as bacc
import concourse.tile as tile
from concourse import mybir
from concourse.bass_utils import run_bass_kernel_spmd

M_ROUTERS = 4
C = 1024
D = 64
B = 8
S = 2048
NCORES = 8
NPAIR = M_ROUTERS // 2   # routers packed two per 128-wide matmul
KCH = C // 128           # 8 contraction chunks for the down matmul
ST_DOWN = 512            # moving free dim for down matmuls (1 PSUM bank)
N_ST = S // ST_DOWN      # 4
TB = 128                 # token block (up matmul stationary M)
N_TB = S // TB           # 16
CC = 512                 # c chunk for up matmuls (1 PSUM bank)
N_CC = C // CC           # 2
BLK = 2                  # token blocks per staging tile -> 1 MiB output DMAs

BF16 = mybir.dt.bfloat16
F32 = mybir.dt.float32

_GRAPH = None


def _build(reps=1):
    nc = bacc.Bacc(None, target_bir_lowering=False, debug=False)
    # xt packed by s-tile: [N_ST, 128(c%128), KCH, ST_DOWN]
    xt = nc.declare_dram_parameter("xt", [N_ST, 128, KCH, ST_DOWN], BF16, isOutput=False)
    wd = nc.declare_dram_parameter("wd", [NPAIR, 128, KCH, 128], BF16, isOutput=False)
    wu = nc.declare_dram_parameter("wu", [NPAIR, 128, C], BF16, isOutput=False)
    bi = nc.declare_dram_parameter("bias", [NPAIR, 128, 1], F32, isOutput=False)
    out = nc.declare_dram_parameter("out", [M_ROUTERS, S, C], BF16, isOutput=True)

    with ExitStack() as ctx:
        tc = ctx.enter_context(tile.TileContext(nc))
        const = ctx.enter_context(tc.tile_pool(name="const", bufs=1))
        psum_d = ctx.enter_context(tc.tile_pool(name="psum_d", bufs=2, space="PSUM"))
        psum_u = ctx.enter_context(tc.tile_pool(name="psum_u", bufs=3, space="PSUM"))
        stage = ctx.enter_context(tc.tile_pool(name="stage", bufs=4))

        if reps > 1:
            # benchmark variant: serialize `reps` full executions; the Tile
            # For_i back-edge is an all-engine barrier, so each iteration
            # times like an independent single-shot run.
            loop = ctx.enter_context(tc.For_i(0, reps, 1))

        # x on the SWDGE (gpsimd) ring, weights on the HWDGE (sync) ring so
        # the first s-tile of x and the pair-0 weights land concurrently.
        x_sb = []
        for st in range(N_ST):
            t = const.tile([128, KCH, ST_DOWN], BF16, tag=f"x{st}")
            nc.gpsimd.dma_start(t[:, :, :], xt[st])
            x_sb.append(t)
        wd_sb, wu_sb, bi_sb, z_sb = [], [], [], []
        for p in range(NPAIR):
            t = const.tile([128, KCH, 128], BF16, tag=f"wd{p}")
            nc.sync.dma_start(t[:, :, :], wd[p])
            wd_sb.append(t)
            t = const.tile([128, 1], F32, tag=f"bi{p}")
            nc.sync.dma_start(t[:], bi[p])
            bi_sb.append(t)
            t = const.tile([128, C], BF16, tag=f"wu{p}")
            nc.sync.dma_start(t[:], wu[p])
            wu_sb.append(t)
            t = const.tile([128, S], BF16, tag=f"z{p}")
            z_sb.append(t)

        TB_PER_ST = ST_DOWN // TB  # token blocks per s-tile (4)
        counters = {"copy": 0, "dma": 0}

        GRP = BLK * TB  # tokens per store group (256)

        def do_phase(p, st):
            # down projection for router pair p, s-tile st
            zp = psum_d.tile([128, ST_DOWN], F32, tag="zp", name="zp")
            for k in range(KCH):
                nc.tensor.matmul(
                    zp[:],
                    wd_sb[p][:, k, :],
                    x_sb[st][:, k, :],
                    start=(k == 0),
                    stop=(k == KCH - 1),
                )
            for t0 in range(st * TB_PER_ST // BLK, (st + 1) * TB_PER_ST // BLK):
                # bias + swish for just this store group's tokens, emitted
                # right before the up matmuls that consume it -- keeps ACT's
                # FIFO from stalling copies behind a big Silu.
                g = t0 - st * TB_PER_ST // BLK
                nc.scalar.activation(
                    z_sb[p][:, t0 * GRP:(t0 + 1) * GRP],
                    zp[:, g * GRP:(g + 1) * GRP],
                    mybir.ActivationFunctionType.Silu,
                    bias=bi_sb[p][:],
                )
                stgs = [
                    stage.tile([128, BLK, C], BF16, tag=f"stg{r}", name=f"stg{r}")
                    for r in range(2)
                ]
                for a in range(BLK):
                    j = t0 * BLK + a
                    for r in range(2):
                        lo, hi = 64 * r, 64 * (r + 1)
                        ups = psum_u.tile([128, C], F32, tag="ups", name="ups")
                        for cc in range(N_CC):
                            nc.tensor.matmul(
                                ups[:, cc * CC:(cc + 1) * CC],
                                z_sb[p][lo:hi, j * TB:(j + 1) * TB],
                                wu_sb[p][lo:hi, cc * CC:(cc + 1) * CC],
                                start=True,
                                stop=True,
                            )
                        if counters["copy"] % 2 == 0:
                            nc.vector.tensor_copy(stgs[r][:, a, :], ups[:])
                        else:
                            nc.scalar.copy(stgs[r][:, a, :], ups[:])
                        counters["copy"] += 1
                for r in range(2):
                    m = 2 * p + r
                    dma_eng = nc.sync if counters["dma"] % 2 == 0 else nc.scalar
                    dma_eng.dma_start(
                        out[m, t0 * GRP:(t0 + 1) * GRP, :]
                        .rearrange("(a q) c -> q a c", q=128),
                        stgs[r][:, :, :],
                    )
                    counters["dma"] += 1

        phases = [(p, st) for p in range(NPAIR) for st in range(N_ST)]
        for ph in phases:
            do_phase(*ph)

    nc.finalize()
    return nc


def _get_graph(reps=1):
    global _GRAPH
    if reps != 1:
        return _build(reps)
    if _GRAPH is None:
        _GRAPH = _build()
    return _GRAPH


def _pack_core_inputs(b, x, dw, db, uw, ei):
    """Host-side shard + expert-gather + layout packing for core b."""
    sel = ei[:, b]                                   # [M]
    ar = np.arange(M_ROUTERS)
    wd_sel = dw[ar, sel]                             # [M, C, D]
    bi_sel = db[ar, sel]                             # [M, D]
    wu_sel = uw[ar, sel]                             # [M, D, C]

    # down weights: concat router pair along D -> [C, 128], then chunk C
    # and make the within-chunk c index the leading (partition) dim.
    wd_packed = np.stack(
        [np.concatenate([wd_sel[2 * p], wd_sel[2 * p + 1]], axis=1) for p in range(NPAIR)]
    )                                                # [NPAIR, C, 128]
    wd_packed = (
        wd_packed.reshape(NPAIR, KCH, 128, 128)
        .transpose(0, 2, 1, 3)                       # [NPAIR, 128(c), KCH, 128(d2)]
        .astype(ml_dtypes.bfloat16)
    )
    bi_packed = np.stack(
        [np.concatenate([bi_sel[2 * p], bi_sel[2 * p + 1]]) for p in range(NPAIR)]
    ).reshape(NPAIR, 128, 1).astype(np.float32)
    wu_packed = np.stack(
        [np.concatenate([wu_sel[2 * p], wu_sel[2 * p + 1]], axis=0) for p in range(NPAIR)]
    ).astype(ml_dtypes.bfloat16)                     # [NPAIR, 128(d2), C]

    xtb = np.ascontiguousarray(x[b].T).astype(ml_dtypes.bfloat16)  # [C, S]
    # [C, S] -> [N_ST, 128(c%128), KCH, ST_DOWN]
    xtb = xtb.reshape(KCH, 128, N_ST, ST_DOWN).transpose(2, 1, 0, 3)

    return {
        "xt": np.ascontiguousarray(xtb),
        "wd": np.ascontiguousarray(wd_packed),
        "wu": np.ascontiguousarray(wu_packed),
        "bias": bi_packed,
    }


def _run(inputs, trace=False):
    x = np.asarray(inputs["x"], dtype=np.float32)
    dw = np.asarray(inputs["down_samplers_weights"], dtype=np.float32)
    db = np.asarray(inputs["down_samplers_bias"], dtype=np.float32)
    uw = np.asarray(inputs["up_samplers_weights"], dtype=np.float32)
    ei = np.asarray(inputs["expert_index"]).astype(np.int64)

    nc = _get_graph()
    in_maps = [_pack_core_inputs(b, x, dw, db, uw, ei) for b in range(NCORES)]
    res = run_bass_kernel_spmd(nc, in_maps, core_ids=list(range(NCORES)), trace=trace)
    out = np.stack(
        [res.results[i]["out"].astype(np.float32) for i in range(NCORES)], axis=1
    )
    return out, res


def kernel(x, down_samplers_weights, down_samplers_bias, up_samplers_weights,
           expert_index):
    out, _ = _run(
        {
            "x": x,
            "down_samplers_weights": down_samplers_weights,
            "down_samplers_bias": down_samplers_bias,
            "up_samplers_weights": up_samplers_weights,
            "expert_index": expert_index,
        },
        trace=False,
    )
    return out


# revision 23
# speedup vs baseline: 16.8759x; 2.7628x over previous
"""Trainium2 Bass kernel: AdapterController hard-routing MoE.

Per (router m, batch b): e = expert_index[m, b], then
  u[m, b] = swish(x[b] @ Wd[m, e] + bd[m, e]) @ Wu[m, e].

Strategy (8 NeuronCores): data-parallel over batch (B == 8, one batch row
per core). The expert gather is done host-side -- each core only needs its
4 selected (Wd, bd, Wu) triples, not the full expert tables. Routers are
packed in pairs so the down-projection runs as full 128-wide matmuls:

  down: lhsT = packed Wd chunk [c128, 128]   (two routers' [c,64] concat)
        rhs  = xT chunk        [c128, s512]
        -> psum zT [128, s512] accumulated over 8 c-chunks
  bias + swish fused on ScalarE (Silu activation, per-partition bias),
        chunked per store group so ACT's FIFO never parks copies
  up:   row-packed K=64 matmuls (router A on partitions 0-63, B on 64-127)
        lhsT = zT[64r:64r+64, s128], rhs = Wu[64r:64r+64, c512]
        -> psum u [s128, c512]
  psum f32 -> SBUF bf16 staging (DVE/ACT alternating), 512 KiB stores.
  Output is written bf16 on-device and upcast to f32 on the host after the
  gather (rel err stays ~4e-3, and it halves the dominant HBM write).
"""

import numpy as np
import ml_dtypes
from contextlib import ExitStack

import concourse.bacc as bacc
import concourse.tile as tile
from concourse import mybir
from concourse.bass_utils import run_bass_kernel_spmd

M_ROUTERS = 4
C = 1024
D = 64
B = 8
S = 2048
NCORES = 8
NPAIR = M_ROUTERS // 2   # routers packed two per 128-wide matmul
KCH = C // 128           # 8 contraction chunks for the down matmul
ST_DOWN = 512            # moving free dim for down matmuls (1 PSUM bank)
N_ST = S // ST_DOWN      # 4
TB = 128                 # token block (up matmul stationary M)
N_TB = S // TB           # 16
CC = 512                 # c chunk for up matmuls (1 PSUM bank)
N_CC = C // CC           # 2
BLK = 2                  # token blocks per staging tile -> 1 MiB output DMAs

BF16 = mybir.dt.bfloat16
F32 = mybir.dt.float32

_GRAPH = None


def _build(reps=1):
    nc = bacc.Bacc(None, target_bir_lowering=False, debug=False)
    # xt packed by s-tile: [N_ST, 128(c%128), KCH, ST_DOWN]
    xt = nc.declare_dram_parameter("xt", [N_ST, 128, KCH, ST_DOWN], BF16, isOutput=False)
    wd = nc.declare_dram_parameter("wd", [NPAIR, 128, KCH, 128], BF16, isOutput=False)
    wu = nc.declare_dram_parameter("wu", [NPAIR, 128, C], BF16, isOutput=False)
    bi = nc.declare_dram_parameter("bias", [NPAIR, 128, 1], F32, isOutput=False)
    out = nc.declare_dram_parameter("out", [M_ROUTERS, S, C], BF16, isOutput=True)

    with ExitStack() as ctx:
        tc = ctx.enter_context(tile.TileContext(nc))
        const = ctx.enter_context(tc.tile_pool(name="const", bufs=1))
        psum_d = ctx.enter_context(tc.tile_pool(name="psum_d", bufs=2, space="PSUM"))
        psum_u = ctx.enter_context(tc.tile_pool(name="psum_u", bufs=3, space="PSUM"))
        stage = ctx.enter_context(tc.tile_pool(name="stage", bufs=4))

        if reps > 1:
            # benchmark variant: serialize `reps` full executions; the Tile
            # For_i back-edge is an all-engine barrier, so each iteration
            # times like an independent single-shot run.
            loop = ctx.enter_context(tc.For_i(0, reps, 1))

        # x on the SWDGE (gpsimd) ring, weights on the HWDGE (sync) ring so
        # the first s-tile of x and the pair-0 weights land concurrently.
        x_sb = []
        for st in range(N_ST):
            t = const.tile([128, KCH, ST_DOWN], BF16, tag=f"x{st}")
            nc.gpsimd.dma_start(t[:, :, :], xt[st])
            x_sb.append(t)
        wd_sb, wu_sb, bi_sb, z_sb = [], [], [], []
        for p in range(NPAIR):
            t = const.tile([128, KCH, 128], BF16, tag=f"wd{p}")
            nc.sync.dma_start(t[:, :, :], wd[p])
            wd_sb.append(t)
            t = const.tile([128, 1], F32, tag=f"bi{p}")
            nc.sync.dma_start(t[:], bi[p])
            bi_sb.append(t)
            t = const.tile([128, C], BF16, tag=f"wu{p}")
            nc.sync.dma_start(t[:], wu[p])
            wu_sb.append(t)
            t = const.tile([128, S], BF16, tag=f"z{p}")
            z_sb.append(t)

        TB_PER_ST = ST_DOWN // TB  # token blocks per s-tile (4)
        counters = {"copy": 0, "dma": 0}

        GRP = BLK * TB  # tokens per store group (256)

        def do_down(p, st):
            zp = psum_d.tile([128, ST_DOWN], F32, tag="zp", name="zp")
            for k in range(KCH):
                nc.tensor.matmul(
                    zp[:],
                    wd_sb[p][:, k, :],
                    x_sb[st][:, k, :],
                    start=(k == 0),
                    stop=(k == KCH - 1),
                )
            return zp

        def do_group(p, st, zp, t0):
            if True:
                # bias + swish for just this store group's tokens, emitted
                # right before the up matmuls that consume it -- keeps ACT's
                # FIFO from stalling copies behind a big Silu.
                g = t0 - st * TB_PER_ST // BLK
                nc.scalar.activation(
                    z_sb[p][:, t0 * GRP:(t0 + 1) * GRP],
                    zp[:, g * GRP:(g + 1) * GRP],
                    mybir.ActivationFunctionType.Silu,
                    bias=bi_sb[p][:],
                )
                stgs = [
                    stage.tile([128, BLK, C], BF16, tag=f"stg{r}", name=f"stg{r}")
                    for r in range(2)
                ]
                for a in range(BLK):
                    j = t0 * BLK + a
                    for r in range(2):
                        lo, hi = 64 * r, 64 * (r + 1)
                        ups = psum_u.tile([128, C], F32, tag="ups", name="ups")
                        for cc in range(N_CC):
                            nc.tensor.matmul(
                                ups[:, cc * CC:(cc + 1) * CC],
                                z_sb[p][lo:hi, j * TB:(j + 1) * TB],
                                wu_sb[p][lo:hi, cc * CC:(cc + 1) * CC],
                                start=True,
                                stop=True,
                            )
                        if counters["copy"] % 2 == 0:
                            nc.vector.tensor_copy(stgs[r][:, a, :], ups[:])
                        else:
                            nc.scalar.copy(stgs[r][:, a, :], ups[:])
                        counters["copy"] += 1
                for r in range(2):
                    m = 2 * p + r
                    dma_eng = nc.sync if counters["dma"] % 2 == 0 else nc.scalar
                    dma_eng.dma_start(
                        out[m, t0 * GRP:(t0 + 1) * GRP, :]
                        .rearrange("(a q) c -> q a c", q=128),
                        stgs[r][:, :, :],
                    )
                    counters["dma"] += 1

        phases = [(p, st) for p in range(NPAIR) for st in range(N_ST)]
        GRPS = TB_PER_ST // BLK  # store groups per s-tile (2)
        zp_cur = do_down(*phases[0])
        for i, (p, st) in enumerate(phases):
            base = st * GRPS
            do_group(p, st, zp_cur, base + 0)
            if i + 1 < len(phases):
                zp_next = do_down(*phases[i + 1])
            for g in range(1, GRPS):
                do_group(p, st, zp_cur, base + g)
            if i + 1 < len(phases):
                zp_cur = zp_next

    nc.finalize()
    return nc


def _get_graph(reps=1):
    global _GRAPH
    if reps != 1:
        return _build(reps)
    if _GRAPH is None:
        _GRAPH = _build()
    return _GRAPH


def _pack_core_inputs(b, x, dw, db, uw, ei):
    """Host-side shard + expert-gather + layout packing for core b."""
    sel = ei[:, b]                                   # [M]
    ar = np.arange(M_ROUTERS)
    wd_sel = dw[ar, sel]                             # [M, C, D]
    bi_sel = db[ar, sel]                             # [M, D]
    wu_sel = uw[ar, sel]                             # [M, D, C]

    # down weights: concat router pair along D -> [C, 128], then chunk C
    # and make the within-chunk c index the leading (partition) dim.
    wd_packed = np.stack(
        [np.concatenate([wd_sel[2 * p], wd_sel[2 * p + 1]], axis=1) for p in range(NPAIR)]
    )                                                # [NPAIR, C, 128]
    wd_packed = (
        wd_packed.reshape(NPAIR, KCH, 128, 128)
        .transpose(0, 2, 1, 3)                       # [NPAIR, 128(c), KCH, 128(d2)]
        .astype(ml_dtypes.bfloat16)
    )
    bi_packed = np.stack(
        [np.concatenate([bi_sel[2 * p], bi_sel[2 * p + 1]]) for p in range(NPAIR)]
    ).reshape(NPAIR, 128, 1).astype(np.float32)
    wu_packed = np.stack(
        [np.concatenate([wu_sel[2 * p], wu_sel[2 * p + 1]], axis=0) for p in range(NPAIR)]
    ).astype(ml_dtypes.bfloat16)                     # [NPAIR, 128(d2), C]

    xtb = np.ascontiguousarray(x[b].T).astype(ml_dtypes.bfloat16)  # [C, S]
    # [C, S] -> [N_ST, 128(c%128), KCH, ST_DOWN]
    xtb = xtb.reshape(KCH, 128, N_ST, ST_DOWN).transpose(2, 1, 0, 3)

    return {
        "xt": np.ascontiguousarray(xtb),
        "wd": np.ascontiguousarray(wd_packed),
        "wu": np.ascontiguousarray(wu_packed),
        "bias": bi_packed,
    }


def _run(inputs, trace=False):
    x = np.asarray(inputs["x"], dtype=np.float32)
    dw = np.asarray(inputs["down_samplers_weights"], dtype=np.float32)
    db = np.asarray(inputs["down_samplers_bias"], dtype=np.float32)
    uw = np.asarray(inputs["up_samplers_weights"], dtype=np.float32)
    ei = np.asarray(inputs["expert_index"]).astype(np.int64)

    nc = _get_graph()
    in_maps = [_pack_core_inputs(b, x, dw, db, uw, ei) for b in range(NCORES)]
    res = run_bass_kernel_spmd(nc, in_maps, core_ids=list(range(NCORES)), trace=trace)
    out = np.stack(
        [res.results[i]["out"].astype(np.float32) for i in range(NCORES)], axis=1
    )
    return out, res


def kernel(x, down_samplers_weights, down_samplers_bias, up_samplers_weights,
           expert_index):
    out, _ = _run(
        {
            "x": x,
            "down_samplers_weights": down_samplers_weights,
            "down_samplers_bias": down_samplers_bias,
            "up_samplers_weights": up_samplers_weights,
            "expert_index": expert_index,
        },
        trace=False,
    )
    return out
